# revision 5
# baseline (speedup 1.0000x reference)
"""Trainium2 Bass kernel for GQA attention (prefill), SPMD over 8 NeuronCores.

Sharding: tensor-parallel over heads (4-way) x data-parallel over batch (2-way).
Core c handles batch c//4 and head-group c%4 (8 q-heads / 2 kv-heads of the
32/8 global heads). Each core computes a full [S, D] partial of the output
projection (wo row-parallel); the 4 partials per batch are summed on host
during unsharding.

Device-side layout is fully "transposed": x and all weights are pre-transposed
on host so every matmul contracts over the partition dim with N=512 moving
operands. Scores are computed as S^T [k, q], so softmax needs no on-chip
transpose of the probability matrix; row sums come from an extra ones-column
appended to V; max-subtraction is skipped (inputs are norm-scale, scores/8
stay << 80 so exp cannot overflow).

The [S, S] additive mask is handled by classifying each 128x128 block on host:
  SKIP    (all <= -1e8): probabilities are exactly 0 -> block skipped/memset
  ZERO    (all == 0):    plain exp
  GENERAL (anything else): block (transposed, pre-scaled by sqrt(hd)) is
          shipped to the device and added to raw scores before exp.
This is exact for any mask and optimal for the causal/no-mask cases.
"""

import numpy as np
import ml_dtypes

import concourse.bacc as bacc
import concourse.mybir as mybir
import concourse.tile as tile
from concourse.bass_utils import run_bass_kernel_spmd

# Problem shape (hardcoded per contract).
B, S, D = 2, 2048, 2048
N_HEADS, N_KV_HEADS, HEAD_DIM = 32, 8, 64
TP = 4            # head-group shards
N_CORES = 8
BLK = 128         # block size (partitions)
NB = S // BLK     # 16 blocks along seq
CHUNK = 512       # q-chunk (moving operand width)
NCH = S // CHUNK  # 4 q-chunks
H_LOC = N_HEADS // TP        # 8 q heads per core
KV_LOC = N_KV_HEADS // TP    # 2 kv heads per core
JD = H_LOC * HEAD_DIM        # 512 local head dims
SCALE = 1.0 / float(np.sqrt(HEAD_DIM))

F32 = mybir.dt.float32
BF16 = mybir.dt.bfloat16

COMPUTE = "bf16"  # "bf16" | "f32"

# mask block classes
SKIP, ZERO, GENERAL = 0, 1, 2


def classify_mask(mask: np.ndarray):
    """Classify each [BLK, BLK] block; return (cls, idx, unique_blocks).

    unique_blocks[i] is the transposed mask block scaled by 1/SCALE... no:
    scaled by sqrt(hd) (=1/SCALE) so that exp((raw + m')*SCALE) ==
    exp(raw*SCALE + m).
    """
    cls = np.empty((NB, NB), dtype=np.int64)
    idx = np.full((NB, NB), -1, dtype=np.int64)
    uniq = []
    seen = {}
    for qi in range(NB):
        for kb in range(NB):
            blkm = mask[qi * BLK:(qi + 1) * BLK, kb * BLK:(kb + 1) * BLK]
            if np.all(blkm <= -1e8):
                cls[qi, kb] = SKIP
            elif not np.any(blkm):
                cls[qi, kb] = ZERO
            else:
                cls[qi, kb] = GENERAL
                key = blkm.tobytes()
                if key not in seen:
                    seen[key] = len(uniq)
                    uniq.append(np.ascontiguousarray(blkm.T) / SCALE)
                idx[qi, kb] = seen[key]
    if not uniq:
        uniq.append(np.zeros((BLK, BLK), dtype=np.float32))
    ublk = np.stack(uniq).astype(np.float32)
    return cls, idx, ublk


def build_program(cls, idx, n_ublk, iters=1, compute=COMPUTE):
    DT = BF16 if compute == "bf16" else F32
    nc = bacc.Bacc("TRN2", target_bir_lowering=False, debug=False,
                   num_devices=N_CORES)

    xT = nc.dram_tensor("xT", [D, S], DT, kind="ExternalInput").ap()
    wqT = nc.dram_tensor("wqT", [D, JD], DT, kind="ExternalInput").ap()
    wkT = nc.dram_tensor("wkT", [D, KV_LOC * HEAD_DIM], DT, kind="ExternalInput").ap()
    wvT = nc.dram_tensor("wvT", [D, KV_LOC * HEAD_DIM], DT, kind="ExternalInput").ap()
    woT = nc.dram_tensor("woT", [JD, D], DT, kind="ExternalInput").ap()
    identD = nc.dram_tensor("identD", [BLK, HEAD_DIM], DT, kind="ExternalInput").ap()
    maskT = nc.dram_tensor("maskT", [n_ublk, BLK, BLK], F32, kind="ExternalInput").ap()
    out = nc.dram_tensor("out", [S, D], F32, kind="ExternalOutput").ap()

    with tile.TileContext(nc) as tc:
        with (
            tc.tile_pool(name="wpool", bufs=1) as wp,      # resident weights/consts
            tc.tile_pool(name="kvpool", bufs=1) as kvp,    # resident KT/V across chunks
            tc.tile_pool(name="spool", bufs=3) as sp,      # streaming xt
            tc.tile_pool(name="qpool", bufs=4) as qp,      # QT per chunk
            tc.tile_pool(name="ppool", bufs=4) as pp,      # P tiles
            tc.tile_pool(name="mpool", bufs=4) as mp,      # misc small
            tc.tile_pool(name="opool", bufs=3) as op,      # out staging
            tc.tile_pool(name="psA", bufs=6, space="PSUM") as psA,
            tc.tile_pool(name="psPV", bufs=1, space="PSUM") as psPV,
            tc.tile_pool(name="psV", bufs=1, space="PSUM") as psV,
        ):
            def body():
                # ---- resident loads ----
                wq_sb = []
                wk_sb = []
                wv_sb = []
                for db in range(NB):
                    t = wp.tile([BLK, JD], DT, tag=f"wq{db}", name=f"wq{db}")
                    nc.sync.dma_start(t[:, :], wqT[db * BLK:(db + 1) * BLK, :])
                    wq_sb.append(t)
                    t = wp.tile([BLK, KV_LOC * HEAD_DIM], DT, tag=f"wk{db}", name=f"wk{db}")
                    nc.sync.dma_start(t[:, :], wkT[db * BLK:(db + 1) * BLK, :])
                    wk_sb.append(t)
                    t = wp.tile([BLK, KV_LOC * HEAD_DIM], DT, tag=f"wv{db}", name=f"wv{db}")
                    nc.sync.dma_start(t[:, :], wvT[db * BLK:(db + 1) * BLK, :])
                    wv_sb.append(t)
                wo_sb = []
                for jt in range(JD // BLK):
                    t = wp.tile([BLK, D], DT, tag=f"wo{jt}", name=f"wo{jt}")
                    nc.sync.dma_start(t[:, :], woT[jt * BLK:(jt + 1) * BLK, :])
                    wo_sb.append(t)
                ident = wp.tile([BLK, HEAD_DIM], DT, tag="ident", name="ident")
                nc.sync.dma_start(ident[:, :], identD)
                mk_sb = []
                for i in range(n_ublk):
                    t = wp.tile([BLK, BLK], F32, tag=f"mk{i}", name=f"mk{i}")
                    nc.sync.dma_start(t[:, :], maskT[i, :, :])
                    mk_sb.append(t)

                # KT duplicated per base: KT_bank[kv] rows 0:64 and 64:128 both
                # hold kv-head kv's K^T, so lhsT/rhs partition bases can match.
                KT_bank = [kvp.tile([BLK, S], DT, tag=f"ktb{kv}", name=f"ktb{kv}") for kv in range(KV_LOC)]
                # V tiles per (kv, k-block): [k 128, hd+1] with ones column.
                V_sb = [[kvp.tile([BLK, HEAD_DIM + 1], DT, tag=f"v{kv}_{kb}", name=f"v{kv}_{kb}")
                         for kb in range(NB)] for kv in range(KV_LOC)]

                attnT = [mp.tile([BLK, CHUNK], DT, tag=f"attnT{jt}", name=f"attnT{jt}")
                         for jt in range(JD // BLK)]

                for c in range(NCH):
                    q0 = c * CHUNK
                    # ---- QKV projection for this q-chunk ----
                    qt_ps = [psA.tile([BLK, CHUNK], F32, tag="mm512", name="mm512")
                             for _ in range(JD // BLK)]
                    kt_ps = psA.tile([BLK, CHUNK], F32, tag="mm512", name="mm512")
                    vt_ps = psA.tile([BLK, CHUNK], F32, tag="mm512", name="mm512")
                    for db in range(NB):
                        xt = sp.tile([BLK, CHUNK], DT, tag="xt", name="xt")
                        nc.sync.dma_start(
                            xt[:, :], xT[db * BLK:(db + 1) * BLK, q0:q0 + CHUNK])
                        st, sp_ = (db == 0), (db == NB - 1)
                        for jt in range(JD // BLK):
                            nc.tensor.matmul(
                                qt_ps[jt][:, :],
                                wq_sb[db][:, jt * BLK:(jt + 1) * BLK],
                                xt[:, :], start=st, stop=sp_)
                        nc.tensor.matmul(kt_ps[:, :], wk_sb[db][:, :], xt[:, :],
                                         start=st, stop=sp_)
                        nc.tensor.matmul(vt_ps[:, :], wv_sb[db][:, :], xt[:, :],
                                         start=st, stop=sp_)

                    # QT chunks to SBUF (bf16 cast)
                    qt_sb = [qp.tile([BLK, CHUNK], DT, tag=f"qt{jt}", name=f"qt{jt}")
                             for jt in range(JD // BLK)]
                    for jt in range(JD // BLK):
                        nc.vector.tensor_copy(qt_sb[jt][:, :], qt_ps[jt][:, :])
                    # KT: copy rows to both bases of the dup bank
                    for kv in range(KV_LOC):
                        r0 = kv * HEAD_DIM
                        nc.vector.tensor_copy(
                            KT_bank[kv][0:HEAD_DIM, q0:q0 + CHUNK],
                            kt_ps[r0:r0 + HEAD_DIM, :])
                        nc.sync.dma_start(
                            KT_bank[kv][HEAD_DIM:2 * HEAD_DIM, q0:q0 + CHUNK],
                            KT_bank[kv][0:HEAD_DIM, q0:q0 + CHUNK])
                    # V: VT chunk -> SBUF staging -> PE transpose per k-block
                    vt_stage = mp.tile([BLK, CHUNK], DT, tag="vt_stage", name="vt_stage")
                    nc.vector.tensor_copy(vt_stage[:, :], vt_ps[:, :])
                    for kv in range(KV_LOC):
                        r0 = kv * HEAD_DIM
                        for kk in range(CHUNK // BLK):
                            kb = c * (CHUNK // BLK) + kk
                            v_ps = psV.tile([BLK, HEAD_DIM], DT, tag="vtr", name="vtr")
                            nc.tensor.transpose(
                                v_ps[:, :],
                                vt_stage[r0:r0 + HEAD_DIM, kk * BLK:(kk + 1) * BLK],
                                ident[r0:r0 + HEAD_DIM, 0:HEAD_DIM])
                            nc.vector.tensor_copy(V_sb[kv][kb][:, 0:HEAD_DIM],
                                                  v_ps[:, :])
                            nc.vector.memset(V_sb[kv][kb][:, HEAD_DIM:HEAD_DIM + 1], 1.0)

                    # ---- attention for this q-chunk ----
                    qis = list(range(c * (CHUNK // BLK), (c + 1) * (CHUNK // BLK)))
                    for h in range(H_LOC):
                        kv = h // (H_LOC // KV_LOC)
                        jt, jr = h // 2, (h % 2) * HEAD_DIM
                        # k-blocks with any non-skip sub-block
                        kbs = [kb for kb in range(NB)
                               if any(cls[qi, kb] != SKIP for qi in qis)]
                        pv_ps = psPV.tile([HEAD_DIM + 1, CHUNK], F32, tag="pv", name="pv")
                        for n_kb, kb in enumerate(kbs):
                            st_ps = psA.tile([BLK, CHUNK], F32, tag="mm512", name="mm512")
                            nc.tensor.matmul(
                                st_ps[:, :],
                                KT_bank[kv][jr:jr + HEAD_DIM, kb * BLK:(kb + 1) * BLK],
                                qt_sb[jt][jr:jr + HEAD_DIM, :],
                                start=True, stop=True)
                            p = pp.tile([BLK, CHUNK], DT, tag="p", name="p")
                            # per sub-block handling, merging adjacent ZERO runs
                            qloc = 0
                            while qloc < len(qis):
                                qi = qis[qloc]
                                cl = cls[qi, kb]
                                if cl == ZERO:
                                    qe = qloc
                                    while qe + 1 < len(qis) and cls[qis[qe + 1], kb] == ZERO:
                                        qe += 1
                                    sl = slice(qloc * BLK, (qe + 1) * BLK)
                                    nc.scalar.activation(
                                        p[:, sl], st_ps[:, sl],
                                        mybir.ActivationFunctionType.Exp, scale=SCALE)
                                    qloc = qe + 1
                                    continue
                                sl = slice(qloc * BLK, (qloc + 1) * BLK)
                                if cl == SKIP:
                                    nc.vector.memset(p[:, sl], 0.0)
                                else:
                                    nc.vector.tensor_tensor(
                                        out=st_ps[:, sl], in0=st_ps[:, sl],
                                        in1=mk_sb[idx[qi, kb]][:, :],
                                        op=mybir.AluOpType.add)
                                    nc.scalar.activation(
                                        p[:, sl], st_ps[:, sl],
                                        mybir.ActivationFunctionType.Exp, scale=SCALE)
                                qloc += 1
                            nc.tensor.matmul(
                                pv_ps[:, :], V_sb[kv][kb][:, :], p[:, :],
                                start=(n_kb == 0), stop=(n_kb == len(kbs) - 1))
                        # normalize: recip of ones-row, broadcast, multiply
                        recip = mp.tile([1, CHUNK], F32, tag="recip", name="recip")
                        nc.vector.reciprocal(recip[:, :],
                                             pv_ps[HEAD_DIM:HEAD_DIM + 1, :])
                        bc = mp.tile([HEAD_DIM, CHUNK], F32, tag="bc", name="bc")
                        nc.gpsimd.partition_broadcast(bc[:, :], recip[:, :])
                        nc.vector.tensor_tensor(
                            out=attnT[jt][jr:jr + HEAD_DIM, :],
                            in0=pv_ps[0:HEAD_DIM, :], in1=bc[:, :],
                            op=mybir.AluOpType.mult)

                    # ---- output projection for this q-chunk ----
                    for ql in range(CHUNK // BLK):
                        qi = qis[ql]
                        for et in range(D // CHUNK):
                            op_ps = psA.tile([BLK, CHUNK], F32, tag="mm512", name="mm512")
                            for jt in range(JD // BLK):
                                nc.tensor.matmul(
                                    op_ps[:, :],
                                    attnT[jt][:, ql * BLK:(ql + 1) * BLK],
                                    wo_sb[jt][:, et * CHUNK:(et + 1) * CHUNK],
                                    start=(jt == 0), stop=(jt == JD // BLK - 1))
                            o_sb = op.tile([BLK, CHUNK], F32, tag="o", name="o")
                            nc.vector.tensor_copy(o_sb[:, :], op_ps[:, :])
                            nc.sync.dma_start(
                                out[qi * BLK:(qi + 1) * BLK,
                                    et * CHUNK:(et + 1) * CHUNK],
                                o_sb[:, :])

            if iters == 1:
                body()
            else:
                with tc.For_i(0, iters):
                    body()
    nc.compile()
    return nc


def make_in_maps(x, wq, wk, wv, wo, ublk, compute=COMPUTE):
    npdt = ml_dtypes.bfloat16 if compute == "bf16" else np.float32
    ident = np.tile(np.eye(HEAD_DIM, dtype=np.float32), (2, 1)).astype(npdt)
    in_maps = []
    for c in range(N_CORES):
        b, g = c // TP, c % TP
        in_maps.append({
            "xT": np.ascontiguousarray(x[b].T).astype(npdt),
            "wqT": np.ascontiguousarray(wq[g * JD:(g + 1) * JD, :].T).astype(npdt),
            "wkT": np.ascontiguousarray(
                wk[g * KV_LOC * HEAD_DIM:(g + 1) * KV_LOC * HEAD_DIM, :].T).astype(npdt),
            "wvT": np.ascontiguousarray(
                wv[g * KV_LOC * HEAD_DIM:(g + 1) * KV_LOC * HEAD_DIM, :].T).astype(npdt),
            "woT": np.ascontiguousarray(wo[:, g * JD:(g + 1) * JD].T).astype(npdt),
            "identD": ident,
            "maskT": ublk,
        })
    return in_maps


def kernel(x, wq, wk, wv, wo, mask, start_pos):
    x = np.asarray(x, dtype=np.float32)
    wq = np.asarray(wq, dtype=np.float32)
    wk = np.asarray(wk, dtype=np.float32)
    wv = np.asarray(wv, dtype=np.float32)
    wo = np.asarray(wo, dtype=np.float32)
    mask = np.asarray(mask, dtype=np.float32)

    cls, idx, ublk = classify_mask(mask)
    nc = build_program(cls, idx, len(ublk), iters=1)
    in_maps = make_in_maps(x, wq, wk, wv, wo, ublk)
    res = run_bass_kernel_spmd(nc, in_maps, core_ids=list(range(N_CORES)),
                               trace=False)
    out = np.zeros((B, S, D), dtype=np.float32)
    for c in range(N_CORES):
        out[c // TP] += res.results[c]["out"]
    return out


# revision 8
# speedup vs baseline: 1.0715x; 1.0715x over previous
"""Trainium2 Bass kernel for GQA attention (prefill), SPMD over 8 NeuronCores.

Sharding: tensor-parallel over heads (4-way) x data-parallel over batch (2-way).
Core c handles batch c//4 and head-group c%4 (8 q-heads / 2 kv-heads of the
32/8 global heads). Each core computes a full [S, D] partial of the output
projection (wo row-parallel); the 4 partials per batch are summed on host
during unsharding.

Device-side layout is fully "transposed": x and all weights are pre-transposed
on host so every matmul contracts over the partition dim with N=512 moving
operands. Scores are computed as S^T [k, q], so softmax needs no on-chip
transpose of the probability matrix; row sums come from an extra ones-column
appended to V; max-subtraction is skipped (inputs are norm-scale, scores/8
stay << 80 so exp cannot overflow).

The [S, S] additive mask is handled by classifying each 128x128 block on host:
  SKIP    (all <= -1e8): probabilities are exactly 0 -> block skipped/memset
  ZERO    (all == 0):    plain exp
  GENERAL (anything else): block (transposed, pre-scaled by sqrt(hd)) is
          shipped to the device and added to raw scores before exp.
This is exact for any mask and optimal for the causal/no-mask cases.
"""

import numpy as np
import ml_dtypes

import concourse.bacc as bacc
import concourse.mybir as mybir
import concourse.tile as tile
from concourse.bass_utils import run_bass_kernel_spmd

# Problem shape (hardcoded per contract).
B, S, D = 2, 2048, 2048
N_HEADS, N_KV_HEADS, HEAD_DIM = 32, 8, 64
TP = 4            # head-group shards
N_CORES = 8
BLK = 128         # block size (partitions)
NB = S // BLK     # 16 blocks along seq
CHUNK = 512       # q-chunk (moving operand width)
NCH = S // CHUNK  # 4 q-chunks
H_LOC = N_HEADS // TP        # 8 q heads per core
KV_LOC = N_KV_HEADS // TP    # 2 kv heads per core
JD = H_LOC * HEAD_DIM        # 512 local head dims
SCALE = 1.0 / float(np.sqrt(HEAD_DIM))

F32 = mybir.dt.float32
BF16 = mybir.dt.bfloat16

COMPUTE = "bf16"  # "bf16" | "f32"

# mask block classes
SKIP, ZERO, GENERAL = 0, 1, 2


def classify_mask(mask: np.ndarray):
    """Classify each [BLK, BLK] block; return (cls, idx, unique_blocks).

    unique_blocks[i] is the transposed mask block scaled by 1/SCALE... no:
    scaled by sqrt(hd) (=1/SCALE) so that exp((raw + m')*SCALE) ==
    exp(raw*SCALE + m).
    """
    cls = np.empty((NB, NB), dtype=np.int64)
    idx = np.full((NB, NB), -1, dtype=np.int64)
    uniq = []
    seen = {}
    for qi in range(NB):
        for kb in range(NB):
            blkm = mask[qi * BLK:(qi + 1) * BLK, kb * BLK:(kb + 1) * BLK]
            if np.all(blkm <= -1e8):
                cls[qi, kb] = SKIP
            elif not np.any(blkm):
                cls[qi, kb] = ZERO
            else:
                cls[qi, kb] = GENERAL
                key = blkm.tobytes()
                if key not in seen:
                    seen[key] = len(uniq)
                    uniq.append(np.ascontiguousarray(blkm.T) / SCALE)
                idx[qi, kb] = seen[key]
    if not uniq:
        uniq.append(np.zeros((BLK, BLK), dtype=np.float32))
    ublk = np.stack(uniq).astype(np.float32)
    return cls, idx, ublk


def build_program(cls, idx, n_ublk, iters=1, compute=COMPUTE, phases=("proj", "attn", "out")):
    DT = BF16 if compute == "bf16" else F32
    nc = bacc.Bacc("TRN2", target_bir_lowering=False, debug=False,
                   num_devices=N_CORES)

    xT = nc.dram_tensor("xT", [D, S], DT, kind="ExternalInput").ap()
    wqT = nc.dram_tensor("wqT", [D, JD], DT, kind="ExternalInput").ap()
    wkT = nc.dram_tensor("wkT", [D, KV_LOC * HEAD_DIM], DT, kind="ExternalInput").ap()
    wvT = nc.dram_tensor("wvT", [D, KV_LOC * HEAD_DIM], DT, kind="ExternalInput").ap()
    woT = nc.dram_tensor("woT", [JD, D], DT, kind="ExternalInput").ap()
    identD = nc.dram_tensor("identD", [BLK, HEAD_DIM], DT, kind="ExternalInput").ap()
    maskT = nc.dram_tensor("maskT", [n_ublk, BLK, BLK], F32, kind="ExternalInput").ap()
    out = nc.dram_tensor("out", [S, D], F32, kind="ExternalOutput").ap()

    with tile.TileContext(nc) as tc:
        with (
            tc.tile_pool(name="wpool", bufs=1) as wp,      # resident weights/consts
            tc.tile_pool(name="kvpool", bufs=1) as kvp,    # resident KT/V across chunks
            tc.tile_pool(name="spool", bufs=4) as sp,      # streaming xt
            tc.tile_pool(name="qpool", bufs=4) as qp,      # QT per chunk
            tc.tile_pool(name="ppool", bufs=6) as pp,      # P tiles
            tc.tile_pool(name="mpool", bufs=4) as mp,      # misc small
            tc.tile_pool(name="opool", bufs=3) as op,      # out staging
            tc.tile_pool(name="psA", bufs=6, space="PSUM") as psA,
            tc.tile_pool(name="psPV", bufs=2, space="PSUM") as psPV,
        ):
            def body():
                # ---- resident loads ----
                wq_sb = []
                wk_sb = []
                wv_sb = []
                for db in range(NB):
                    t = wp.tile([BLK, JD], DT, tag=f"wq{db}", name=f"wq{db}")
                    nc.sync.dma_start(t[:, :], wqT[db * BLK:(db + 1) * BLK, :])
                    wq_sb.append(t)
                    t = wp.tile([BLK, KV_LOC * HEAD_DIM], DT, tag=f"wk{db}", name=f"wk{db}")
                    nc.sync.dma_start(t[:, :], wkT[db * BLK:(db + 1) * BLK, :])
                    wk_sb.append(t)
                    t = wp.tile([BLK, KV_LOC * HEAD_DIM], DT, tag=f"wv{db}", name=f"wv{db}")
                    nc.sync.dma_start(t[:, :], wvT[db * BLK:(db + 1) * BLK, :])
                    wv_sb.append(t)
                wo_sb = []
                for jt in range(JD // BLK):
                    t = wp.tile([BLK, D], DT, tag=f"wo{jt}", name=f"wo{jt}")
                    nc.sync.dma_start(t[:, :], woT[jt * BLK:(jt + 1) * BLK, :])
                    wo_sb.append(t)
                ident = wp.tile([BLK, HEAD_DIM], DT, tag="ident", name="ident")
                nc.sync.dma_start(ident[:, :], identD)
                mk_sb = []
                for i in range(n_ublk):
                    t = wp.tile([BLK, BLK], F32, tag=f"mk{i}", name=f"mk{i}")
                    nc.sync.dma_start(t[:, :], maskT[i, :, :])
                    mk_sb.append(t)

                # KT duplicated per base: KT_bank[kv] rows 0:64 and 64:128 both
                # hold kv-head kv's K^T, so lhsT/rhs partition bases can match.
                KT_bank = [kvp.tile([BLK, S], DT, tag=f"ktb{kv}", name=f"ktb{kv}") for kv in range(KV_LOC)]
                # V tiles per (kv, k-block): [k 128, hd+1] with ones column.
                V_sb = [[kvp.tile([BLK, HEAD_DIM + 1], DT, tag=f"v{kv}_{kb}", name=f"v{kv}_{kb}")
                         for kb in range(NB)] for kv in range(KV_LOC)]

                attnT = [mp.tile([BLK, CHUNK], DT, tag=f"attnT{jt}", name=f"attnT{jt}")
                         for jt in range(JD // BLK)]

                for c in range(NCH):
                    q0 = c * CHUNK
                    # ---- QKV projection for this q-chunk ----
                    qt_ps = [psA.tile([BLK, CHUNK], F32, tag="mm512", name="mm512")
                             for _ in range(JD // BLK)]
                    kt_ps = psA.tile([BLK, CHUNK], F32, tag="mm512", name="mm512")
                    vt_ps = psA.tile([BLK, CHUNK], F32, tag="mm512", name="mm512")
                    for db in range(NB):
                        xt = sp.tile([BLK, CHUNK], DT, tag="xt", name="xt")
                        nc.sync.dma_start(
                            xt[:, :], xT[db * BLK:(db + 1) * BLK, q0:q0 + CHUNK])
                        st, sp_ = (db == 0), (db == NB - 1)
                        for jt in range(JD // BLK):
                            nc.tensor.matmul(
                                qt_ps[jt][:, :],
                                wq_sb[db][:, jt * BLK:(jt + 1) * BLK],
                                xt[:, :], start=st, stop=sp_)
                        nc.tensor.matmul(kt_ps[:, :], wk_sb[db][:, :], xt[:, :],
                                         start=st, stop=sp_)
                        nc.tensor.matmul(vt_ps[:, :], wv_sb[db][:, :], xt[:, :],
                                         start=st, stop=sp_)

                    # QT chunks to SBUF (bf16 cast)
                    qt_sb = [qp.tile([BLK, CHUNK], DT, tag=f"qt{jt}", name=f"qt{jt}")
                             for jt in range(JD // BLK)]
                    for jt in range(JD // BLK):
                        nc.vector.tensor_copy(qt_sb[jt][:, :], qt_ps[jt][:, :])
                    # KT: copy rows to both bases of the dup bank
                    for kv in range(KV_LOC):
                        r0 = kv * HEAD_DIM
                        nc.vector.tensor_copy(
                            KT_bank[kv][0:HEAD_DIM, q0:q0 + CHUNK],
                            kt_ps[r0:r0 + HEAD_DIM, :])
                        nc.sync.dma_start(
                            KT_bank[kv][HEAD_DIM:2 * HEAD_DIM, q0:q0 + CHUNK],
                            KT_bank[kv][0:HEAD_DIM, q0:q0 + CHUNK])
                    # V: VT chunk -> SBUF staging -> PE transpose per k-block
                    vt_stage = mp.tile([BLK, CHUNK], DT, tag="vt_stage", name="vt_stage")
                    nc.vector.tensor_copy(vt_stage[:, :], vt_ps[:, :])
                    for kv in range(KV_LOC):
                        r0 = kv * HEAD_DIM
                        for kk in range(CHUNK // BLK):
                            kb = c * (CHUNK // BLK) + kk
                            v_ps = psA.tile([BLK, HEAD_DIM], DT, tag="mm512", name="mm512")
                            nc.tensor.transpose(
                                v_ps[:, :],
                                vt_stage[r0:r0 + HEAD_DIM, kk * BLK:(kk + 1) * BLK],
                                ident[r0:r0 + HEAD_DIM, 0:HEAD_DIM])
                            nc.vector.tensor_copy(V_sb[kv][kb][:, 0:HEAD_DIM],
                                                  v_ps[:, :])
                            nc.vector.memset(V_sb[kv][kb][:, HEAD_DIM:HEAD_DIM + 1], 1.0)

                    # ---- attention for this q-chunk ----
                    qis = list(range(c * (CHUNK // BLK), (c + 1) * (CHUNK // BLK)))
                    if "attn" not in phases:
                        continue
                    for hp in range(H_LOC // 2):
                        kv = (2 * hp) // (H_LOC // KV_LOC)
                        jt = hp
                        kbs = [kb for kb in range(NB)
                               if any(cls[qi, kb] != SKIP for qi in qis)]
                        pv_ps = [psPV.tile([HEAD_DIM + 1, CHUNK], F32, tag="pv", name="pv")
                                 for _ in range(2)]
                        for n_kb, kb in enumerate(kbs):
                            st_ps = []
                            for sub in range(2):
                                jr = sub * HEAD_DIM
                                stp = psA.tile([BLK, CHUNK], F32, tag="mm512", name="mm512")
                                nc.tensor.matmul(
                                    stp[:, :],
                                    KT_bank[kv][jr:jr + HEAD_DIM, kb * BLK:(kb + 1) * BLK],
                                    qt_sb[jt][jr:jr + HEAD_DIM, :],
                                    start=True, stop=True)
                                st_ps.append(stp)
                            ps_ = []
                            for sub in range(2):
                                stp = st_ps[sub]
                                p = pp.tile([BLK, CHUNK], DT, tag="p", name="p")
                                qloc = 0
                                while qloc < len(qis):
                                    qi = qis[qloc]
                                    cl = cls[qi, kb]
                                    if cl == ZERO:
                                        qe = qloc
                                        while qe + 1 < len(qis) and cls[qis[qe + 1], kb] == ZERO:
                                            qe += 1
                                        sl = slice(qloc * BLK, (qe + 1) * BLK)
                                        nc.scalar.activation(
                                            p[:, sl], stp[:, sl],
                                            mybir.ActivationFunctionType.Exp, scale=SCALE)
                                        qloc = qe + 1
                                        continue
                                    sl = slice(qloc * BLK, (qloc + 1) * BLK)
                                    if cl == SKIP:
                                        nc.vector.memset(p[:, sl], 0.0)
                                    else:
                                        nc.vector.tensor_tensor(
                                            out=stp[:, sl], in0=stp[:, sl],
                                            in1=mk_sb[idx[qi, kb]][:, :],
                                            op=mybir.AluOpType.add)
                                        nc.scalar.activation(
                                            p[:, sl], stp[:, sl],
                                            mybir.ActivationFunctionType.Exp, scale=SCALE)
                                    qloc += 1
                                ps_.append(p)
                            for sub in range(2):
                                nc.tensor.matmul(
                                    pv_ps[sub][:, :], V_sb[kv][kb][:, :], ps_[sub][:, :],
                                    start=(n_kb == 0), stop=(n_kb == len(kbs) - 1))
                        for sub in range(2):
                            jr = sub * HEAD_DIM
                            recip = mp.tile([1, CHUNK], F32, tag="recip", name="recip")
                            nc.vector.reciprocal(recip[:, :],
                                                 pv_ps[sub][HEAD_DIM:HEAD_DIM + 1, :])
                            bc = mp.tile([HEAD_DIM, CHUNK], F32, tag="bc", name="bc")
                            nc.gpsimd.partition_broadcast(bc[:, :], recip[:, :])
                            nc.vector.tensor_tensor(
                                out=attnT[jt][jr:jr + HEAD_DIM, :],
                                in0=pv_ps[sub][0:HEAD_DIM, :], in1=bc[:, :],
                                op=mybir.AluOpType.mult)

                    # ---- output projection for this q-chunk ----
                    if "out" not in phases:
                        continue
                    for ql in range(CHUNK // BLK):
                        qi = qis[ql]
                        for et in range(D // CHUNK):
                            op_ps = psA.tile([BLK, CHUNK], F32, tag="mm512", name="mm512")
                            for jt in range(JD // BLK):
                                nc.tensor.matmul(
                                    op_ps[:, :],
                                    attnT[jt][:, ql * BLK:(ql + 1) * BLK],
                                    wo_sb[jt][:, et * CHUNK:(et + 1) * CHUNK],
                                    start=(jt == 0), stop=(jt == JD // BLK - 1))
                            o_sb = op.tile([BLK, CHUNK], F32, tag="o", name="o")
                            nc.vector.tensor_copy(o_sb[:, :], op_ps[:, :])
                            nc.sync.dma_start(
                                out[qi * BLK:(qi + 1) * BLK,
                                    et * CHUNK:(et + 1) * CHUNK],
                                o_sb[:, :])

            if iters == 1:
                body()
            else:
                with tc.For_i(0, iters):
                    body()
    nc.compile()
    return nc


def make_in_maps(x, wq, wk, wv, wo, ublk, compute=COMPUTE):
    npdt = ml_dtypes.bfloat16 if compute == "bf16" else np.float32
    ident = np.tile(np.eye(HEAD_DIM, dtype=np.float32), (2, 1)).astype(npdt)
    in_maps = []
    for c in range(N_CORES):
        b, g = c // TP, c % TP
        in_maps.append({
            "xT": np.ascontiguousarray(x[b].T).astype(npdt),
            "wqT": np.ascontiguousarray(wq[g * JD:(g + 1) * JD, :].T).astype(npdt),
            "wkT": np.ascontiguousarray(
                wk[g * KV_LOC * HEAD_DIM:(g + 1) * KV_LOC * HEAD_DIM, :].T).astype(npdt),
            "wvT": np.ascontiguousarray(
                wv[g * KV_LOC * HEAD_DIM:(g + 1) * KV_LOC * HEAD_DIM, :].T).astype(npdt),
            "woT": np.ascontiguousarray(wo[:, g * JD:(g + 1) * JD].T).astype(npdt),
            "identD": ident,
            "maskT": ublk,
        })
    return in_maps


def kernel(x, wq, wk, wv, wo, mask, start_pos):
    x = np.asarray(x, dtype=np.float32)
    wq = np.asarray(wq, dtype=np.float32)
    wk = np.asarray(wk, dtype=np.float32)
    wv = np.asarray(wv, dtype=np.float32)
    wo = np.asarray(wo, dtype=np.float32)
    mask = np.asarray(mask, dtype=np.float32)

    cls, idx, ublk = classify_mask(mask)
    nc = build_program(cls, idx, len(ublk), iters=1)
    in_maps = make_in_maps(x, wq, wk, wv, wo, ublk)
    res = run_bass_kernel_spmd(nc, in_maps, core_ids=list(range(N_CORES)),
                               trace=False)
    out = np.zeros((B, S, D), dtype=np.float32)
    for c in range(N_CORES):
        out[c // TP] += res.results[c]["out"]
    return out


# revision 10
# speedup vs baseline: 1.0812x; 1.0090x over previous
"""Trainium2 Bass kernel for GQA attention (prefill), SPMD over 8 NeuronCores.

Sharding: tensor-parallel over heads (4-way) x data-parallel over batch (2-way).
Core c handles batch c//4 and head-group c%4 (8 q-heads / 2 kv-heads of the
32/8 global heads). Each core computes a full [S, D] partial of the output
projection (wo row-parallel); the 4 partials per batch are summed on host
during unsharding.

Device-side layout is fully "transposed": x and all weights are pre-transposed
on host so every matmul contracts over the partition dim with N=512 moving
operands. Scores are computed as S^T [k, q], so softmax needs no on-chip
transpose of the probability matrix; row sums come from an extra ones-column
appended to V; max-subtraction is skipped (inputs are norm-scale, scores/8
stay << 80 so exp cannot overflow).

The [S, S] additive mask is handled by classifying each 128x128 block on host:
  SKIP    (all <= -1e8): probabilities are exactly 0 -> block skipped/memset
  ZERO    (all == 0):    plain exp
  GENERAL (anything else): block (transposed, pre-scaled by sqrt(hd)) is
          shipped to the device and added to raw scores before exp.
This is exact for any mask and optimal for the causal/no-mask cases.
"""

import numpy as np
import ml_dtypes

import concourse.bacc as bacc
import concourse.mybir as mybir
import concourse.tile as tile
from concourse.bass_utils import run_bass_kernel_spmd

# Problem shape (hardcoded per contract).
B, S, D = 2, 2048, 2048
N_HEADS, N_KV_HEADS, HEAD_DIM = 32, 8, 64
TP = 4            # head-group shards
N_CORES = 8
BLK = 128         # block size (partitions)
NB = S // BLK     # 16 blocks along seq
CHUNK = 512       # q-chunk (moving operand width)
NCH = S // CHUNK  # 4 q-chunks
H_LOC = N_HEADS // TP        # 8 q heads per core
KV_LOC = N_KV_HEADS // TP    # 2 kv heads per core
JD = H_LOC * HEAD_DIM        # 512 local head dims
SCALE = 1.0 / float(np.sqrt(HEAD_DIM))

F32 = mybir.dt.float32
BF16 = mybir.dt.bfloat16

COMPUTE = "bf16"  # "bf16" | "f32"

# mask block classes
SKIP, ZERO, GENERAL = 0, 1, 2


def classify_mask(mask: np.ndarray):
    """Classify each [BLK, BLK] block; return (cls, idx, unique_blocks).

    unique_blocks[i] is the transposed mask block scaled by 1/SCALE... no:
    scaled by sqrt(hd) (=1/SCALE) so that exp((raw + m')*SCALE) ==
    exp(raw*SCALE + m).
    """
    cls = np.empty((NB, NB), dtype=np.int64)
    idx = np.full((NB, NB), -1, dtype=np.int64)
    uniq = []
    seen = {}
    for qi in range(NB):
        for kb in range(NB):
            blkm = mask[qi * BLK:(qi + 1) * BLK, kb * BLK:(kb + 1) * BLK]
            if np.all(blkm <= -1e8):
                cls[qi, kb] = SKIP
            elif not np.any(blkm):
                cls[qi, kb] = ZERO
            else:
                cls[qi, kb] = GENERAL
                key = blkm.tobytes()
                if key not in seen:
                    seen[key] = len(uniq)
                    uniq.append(np.ascontiguousarray(blkm.T) / SCALE)
                idx[qi, kb] = seen[key]
    if not uniq:
        uniq.append(np.zeros((BLK, BLK), dtype=np.float32))
    ublk = np.stack(uniq).astype(np.float32)
    return cls, idx, ublk


def build_program(cls, idx, n_ublk, iters=1, compute=COMPUTE, phases=("proj", "attn", "out")):
    DT = BF16 if compute == "bf16" else F32
    nc = bacc.Bacc("TRN2", target_bir_lowering=False, debug=False,
                   num_devices=N_CORES)

    xT = nc.dram_tensor("xT", [D, S], DT, kind="ExternalInput").ap()
    wqT = nc.dram_tensor("wqT", [D, JD], DT, kind="ExternalInput").ap()
    wkT = nc.dram_tensor("wkT", [D, KV_LOC * HEAD_DIM], DT, kind="ExternalInput").ap()
    wvT = nc.dram_tensor("wvT", [D, KV_LOC * HEAD_DIM], DT, kind="ExternalInput").ap()
    woT = nc.dram_tensor("woT", [JD, D], DT, kind="ExternalInput").ap()
    identD = nc.dram_tensor("identD", [BLK, HEAD_DIM], DT, kind="ExternalInput").ap()
    maskT = nc.dram_tensor("maskT", [n_ublk, BLK, BLK], F32, kind="ExternalInput").ap()
    out = nc.dram_tensor("out", [S, D], F32, kind="ExternalOutput").ap()

    with tile.TileContext(nc) as tc:
        with (
            tc.tile_pool(name="wpool", bufs=1) as wp,      # resident weights/consts
            tc.tile_pool(name="kvpool", bufs=1) as kvp,    # resident KT/V across chunks
            tc.tile_pool(name="spool", bufs=4) as sp,      # streaming xt
            tc.tile_pool(name="qpool", bufs=4) as qp,      # QT per chunk
            tc.tile_pool(name="ppool", bufs=6) as pp,      # P tiles
            tc.tile_pool(name="mpool", bufs=4) as mp,      # misc small
            tc.tile_pool(name="opool", bufs=3) as op,      # out staging
            tc.tile_pool(name="psA", bufs=6, space="PSUM") as psA,
            tc.tile_pool(name="psPV", bufs=2, space="PSUM") as psPV,
        ):
            def body():
                # ---- resident loads ----
                wq_sb = []
                wk_sb = []
                wv_sb = []
                for db in range(NB):
                    t = wp.tile([BLK, JD], DT, tag=f"wq{db}", name=f"wq{db}")
                    nc.sync.dma_start(t[:, :], wqT[db * BLK:(db + 1) * BLK, :])
                    wq_sb.append(t)
                    t = wp.tile([BLK, KV_LOC * HEAD_DIM], DT, tag=f"wk{db}", name=f"wk{db}")
                    nc.sync.dma_start(t[:, :], wkT[db * BLK:(db + 1) * BLK, :])
                    wk_sb.append(t)
                    t = wp.tile([BLK, KV_LOC * HEAD_DIM], DT, tag=f"wv{db}", name=f"wv{db}")
                    nc.sync.dma_start(t[:, :], wvT[db * BLK:(db + 1) * BLK, :])
                    wv_sb.append(t)
                wo_sb = []
                for jt in range(JD // BLK):
                    t = wp.tile([BLK, D], DT, tag=f"wo{jt}", name=f"wo{jt}")
                    nc.scalar.dma_start(t[:, :], woT[jt * BLK:(jt + 1) * BLK, :])
                    wo_sb.append(t)
                ident = wp.tile([BLK, HEAD_DIM], DT, tag="ident", name="ident")
                nc.scalar.dma_start(ident[:, :], identD)
                mk_sb = []
                for i in range(n_ublk):
                    t = wp.tile([BLK, BLK], F32, tag=f"mk{i}", name=f"mk{i}")
                    nc.scalar.dma_start(t[:, :], maskT[i, :, :])
                    mk_sb.append(t)

                # KT duplicated per base: KT_bank[kv] rows 0:64 and 64:128 both
                # hold kv-head kv's K^T, so lhsT/rhs partition bases can match.
                KT_bank = [kvp.tile([BLK, S], DT, tag=f"ktb{kv}", name=f"ktb{kv}") for kv in range(KV_LOC)]
                # V tiles per (kv, k-block): [k 128, hd+1] with ones column.
                V_sb = [[kvp.tile([BLK, HEAD_DIM + 1], DT, tag=f"v{kv}_{kb}", name=f"v{kv}_{kb}")
                         for kb in range(NB)] for kv in range(KV_LOC)]

                attnT = [mp.tile([BLK, CHUNK], DT, tag=f"attnT{jt}", name=f"attnT{jt}")
                         for jt in range(JD // BLK)]

                for c in range(NCH):
                    q0 = c * CHUNK
                    # ---- QKV projection for this q-chunk ----
                    qt_ps = [psA.tile([BLK, CHUNK], F32, tag="mm512", name="mm512")
                             for _ in range(JD // BLK)]
                    kt_ps = psA.tile([BLK, CHUNK], F32, tag="mm512", name="mm512")
                    vt_ps = psA.tile([BLK, CHUNK], F32, tag="mm512", name="mm512")
                    for db in range(NB):
                        xt = sp.tile([BLK, CHUNK], DT, tag="xt", name="xt")
                        nc.sync.dma_start(
                            xt[:, :], xT[db * BLK:(db + 1) * BLK, q0:q0 + CHUNK])
                        st, sp_ = (db == 0), (db == NB - 1)
                        for jt in range(JD // BLK):
                            nc.tensor.matmul(
                                qt_ps[jt][:, :],
                                wq_sb[db][:, jt * BLK:(jt + 1) * BLK],
                                xt[:, :], start=st, stop=sp_)
                        nc.tensor.matmul(kt_ps[:, :], wk_sb[db][:, :], xt[:, :],
                                         start=st, stop=sp_)
                        nc.tensor.matmul(vt_ps[:, :], wv_sb[db][:, :], xt[:, :],
                                         start=st, stop=sp_)

                    # QT chunks to SBUF (bf16 cast)
                    qt_sb = [qp.tile([BLK, CHUNK], DT, tag=f"qt{jt}", name=f"qt{jt}")
                             for jt in range(JD // BLK)]
                    for jt in range(JD // BLK):
                        nc.vector.tensor_copy(qt_sb[jt][:, :], qt_ps[jt][:, :])
                    # KT: copy rows to both bases of the dup bank
                    for kv in range(KV_LOC):
                        r0 = kv * HEAD_DIM
                        nc.vector.tensor_copy(
                            KT_bank[kv][0:HEAD_DIM, q0:q0 + CHUNK],
                            kt_ps[r0:r0 + HEAD_DIM, :])
                        nc.sync.dma_start(
                            KT_bank[kv][HEAD_DIM:2 * HEAD_DIM, q0:q0 + CHUNK],
                            KT_bank[kv][0:HEAD_DIM, q0:q0 + CHUNK])
                    # V: VT chunk -> SBUF staging -> PE transpose per k-block
                    vt_stage = mp.tile([BLK, CHUNK], DT, tag="vt_stage", name="vt_stage")
                    nc.vector.tensor_copy(vt_stage[:, :], vt_ps[:, :])
                    for kv in range(KV_LOC):
                        r0 = kv * HEAD_DIM
                        for kk in range(CHUNK // BLK):
                            kb = c * (CHUNK // BLK) + kk
                            v_ps = psA.tile([BLK, HEAD_DIM], DT, tag="mm512", name="mm512")
                            nc.tensor.transpose(
                                v_ps[:, :],
                                vt_stage[r0:r0 + HEAD_DIM, kk * BLK:(kk + 1) * BLK],
                                ident[r0:r0 + HEAD_DIM, 0:HEAD_DIM])
                            nc.vector.tensor_copy(V_sb[kv][kb][:, 0:HEAD_DIM],
                                                  v_ps[:, :])
                            nc.vector.memset(V_sb[kv][kb][:, HEAD_DIM:HEAD_DIM + 1], 1.0)

                    # ---- attention for this q-chunk ----
                    qis = list(range(c * (CHUNK // BLK), (c + 1) * (CHUNK // BLK)))
                    if "attn" not in phases:
                        continue
                    for hp in range(H_LOC // 2):
                        kv = (2 * hp) // (H_LOC // KV_LOC)
                        jt = hp
                        kbs = [kb for kb in range(NB)
                               if any(cls[qi, kb] != SKIP for qi in qis)]
                        pv_ps = [psPV.tile([HEAD_DIM + 1, CHUNK], F32, tag="pv", name="pv")
                                 for _ in range(2)]
                        for n_kb, kb in enumerate(kbs):
                            # valid q sub-blocks form a suffix under causal
                            # masks; compute only columns [off:] when so.
                            nsk = [cls[qi, kb] != SKIP for qi in qis]
                            if n_kb > 0 and all(nsk[i] or not any(nsk[i:])
                                                for i in range(len(nsk))):
                                off = nsk.index(True) * BLK
                            else:
                                off = 0  # first kb / non-suffix: compute all
                            st_ps = []
                            for sub in range(2):
                                jr = sub * HEAD_DIM
                                stp = psA.tile([BLK, CHUNK], F32, tag="mm512", name="mm512")
                                nc.tensor.matmul(
                                    stp[:, off:],
                                    KT_bank[kv][jr:jr + HEAD_DIM, kb * BLK:(kb + 1) * BLK],
                                    qt_sb[jt][jr:jr + HEAD_DIM, off:],
                                    start=True, stop=True)
                                st_ps.append(stp)
                            ps_ = []
                            for sub in range(2):
                                stp = st_ps[sub]
                                p = pp.tile([BLK, CHUNK], DT, tag="p", name="p")
                                qloc = off // BLK
                                while qloc < len(qis):
                                    qi = qis[qloc]
                                    cl = cls[qi, kb]
                                    if cl == ZERO:
                                        qe = qloc
                                        while qe + 1 < len(qis) and cls[qis[qe + 1], kb] == ZERO:
                                            qe += 1
                                        sl = slice(qloc * BLK, (qe + 1) * BLK)
                                        nc.scalar.activation(
                                            p[:, sl], stp[:, sl],
                                            mybir.ActivationFunctionType.Exp, scale=SCALE)
                                        qloc = qe + 1
                                        continue
                                    sl = slice(qloc * BLK, (qloc + 1) * BLK)
                                    if cl == SKIP:
                                        nc.vector.memset(p[:, sl], 0.0)
                                    else:
                                        nc.vector.tensor_tensor(
                                            out=stp[:, sl], in0=stp[:, sl],
                                            in1=mk_sb[idx[qi, kb]][:, :],
                                            op=mybir.AluOpType.add)
                                        nc.scalar.activation(
                                            p[:, sl], stp[:, sl],
                                            mybir.ActivationFunctionType.Exp, scale=SCALE)
                                    qloc += 1
                                ps_.append(p)
                            for sub in range(2):
                                nc.tensor.matmul(
                                    pv_ps[sub][:, off:], V_sb[kv][kb][:, :],
                                    ps_[sub][:, off:],
                                    start=(n_kb == 0), stop=(n_kb == len(kbs) - 1))
                        for sub in range(2):
                            jr = sub * HEAD_DIM
                            recip = mp.tile([1, CHUNK], F32, tag="recip", name="recip")
                            nc.vector.reciprocal(recip[:, :],
                                                 pv_ps[sub][HEAD_DIM:HEAD_DIM + 1, :])
                            bc = mp.tile([HEAD_DIM, CHUNK], F32, tag="bc", name="bc")
                            nc.gpsimd.partition_broadcast(bc[:, :], recip[:, :])
                            nc.vector.tensor_tensor(
                                out=attnT[jt][jr:jr + HEAD_DIM, :],
                                in0=pv_ps[sub][0:HEAD_DIM, :], in1=bc[:, :],
                                op=mybir.AluOpType.mult)

                    # ---- output projection for this q-chunk ----
                    if "out" not in phases:
                        continue
                    for ql in range(CHUNK // BLK):
                        qi = qis[ql]
                        for et in range(D // CHUNK):
                            op_ps = psA.tile([BLK, CHUNK], F32, tag="mm512", name="mm512")
                            for jt in range(JD // BLK):
                                nc.tensor.matmul(
                                    op_ps[:, :],
                                    attnT[jt][:, ql * BLK:(ql + 1) * BLK],
                                    wo_sb[jt][:, et * CHUNK:(et + 1) * CHUNK],
                                    start=(jt == 0), stop=(jt == JD // BLK - 1))
                            o_sb = op.tile([BLK, CHUNK], F32, tag="o", name="o")
                            nc.vector.tensor_copy(o_sb[:, :], op_ps[:, :])
                            nc.sync.dma_start(
                                out[qi * BLK:(qi + 1) * BLK,
                                    et * CHUNK:(et + 1) * CHUNK],
                                o_sb[:, :])

            if iters == 1:
                body()
            else:
                with tc.For_i(0, iters):
                    body()
    nc.compile()
    return nc


def make_in_maps(x, wq, wk, wv, wo, ublk, compute=COMPUTE):
    npdt = ml_dtypes.bfloat16 if compute == "bf16" else np.float32
    ident = np.tile(np.eye(HEAD_DIM, dtype=np.float32), (2, 1)).astype(npdt)
    in_maps = []
    for c in range(N_CORES):
        b, g = c // TP, c % TP
        in_maps.append({
            "xT": np.ascontiguousarray(x[b].T).astype(npdt),
            "wqT": np.ascontiguousarray(wq[g * JD:(g + 1) * JD, :].T).astype(npdt),
            "wkT": np.ascontiguousarray(
                wk[g * KV_LOC * HEAD_DIM:(g + 1) * KV_LOC * HEAD_DIM, :].T).astype(npdt),
            "wvT": np.ascontiguousarray(
                wv[g * KV_LOC * HEAD_DIM:(g + 1) * KV_LOC * HEAD_DIM, :].T).astype(npdt),
            "woT": np.ascontiguousarray(wo[:, g * JD:(g + 1) * JD].T).astype(npdt),
            "identD": ident,
            "maskT": ublk,
        })
    return in_maps


def kernel(x, wq, wk, wv, wo, mask, start_pos):
    x = np.asarray(x, dtype=np.float32)
    wq = np.asarray(wq, dtype=np.float32)
    wk = np.asarray(wk, dtype=np.float32)
    wv = np.asarray(wv, dtype=np.float32)
    wo = np.asarray(wo, dtype=np.float32)
    mask = np.asarray(mask, dtype=np.float32)

    cls, idx, ublk = classify_mask(mask)
    nc = build_program(cls, idx, len(ublk), iters=1)
    in_maps = make_in_maps(x, wq, wk, wv, wo, ublk)
    res = run_bass_kernel_spmd(nc, in_maps, core_ids=list(range(N_CORES)),
                               trace=False)
    out = np.zeros((B, S, D), dtype=np.float32)
    for c in range(N_CORES):
        out[c // TP] += res.results[c]["out"]
    return out
